# revision 18
# baseline (speedup 1.0000x reference)
"""Batched Kalman filter for Trainium2 (Bass), 8-core data parallel.

The reference filter's P/K evolution is data- and batch-independent, so the
per-step gains can be computed on the host. When every per-step update matrix
is a scalar multiple of the identity (true for the shipped identity
parameters), the whole filter collapses to

    out[b] = W @ y[b]        W[t, s] = b_s * prod_{r=s+1..t} a_r   (lower-tri)

with a_t = 1 - k_t, b_t = k_t from the scalar gain recursion. On device this
is a single [64, 64] weight matmul applied per batch element.

This problem is HBM-bandwidth bound, so the kernel optimizes DMA traffic:

* Inputs/outputs move as bf16 (host converts; rel tolerance is 2e-2 and the
  bf16 round-trip costs ~0.7% rms), halving HBM bytes vs fp32.
* The input load keeps DMA elements at 512B (full descriptor efficiency) by
  packing E=4 consecutive time-rows of a batch element into one partition.
  The matmul contraction over time is then split into E accumulating passes
  with per-phase block-diagonal weights.
* The E passes leave only 32 of 128 contraction rows per matmul, so the PE
  array is row-tiled: the slab's four pair-blocks live in partition strips
  0-31/32-63/64-95/96-127, and four 32-row matmuls stream concurrently in
  the four PE sub-array strips (tile_position is inferred from the operands'
  base partitions).
* The store is a straight sequential copy of the time-major result (8KB
  contiguous runs -> full bandwidth); the host undoes the layout permute
  during the gather/convert step it does anyway.
"""

import numpy as np
import ml_dtypes

B = 16384
NCORES = 8
BS = B // NCORES          # 2048 batch rows per core

T = 64
D = 64

E = 4                     # time-rows packed per DMA element (E*D*2 = 512B)
NG = T // E               # contraction groups per batch lane
NR = 4                    # PE row strips (pair-blocks per slab)

_CACHE = {}

SLAB = 128                # batch rows per slab
NPAIR = SLAB // 2         # batch pairs per slab
BPAIR = NPAIR // NR       # pairs per strip (16)
SLOT = BPAIR * E * D      # input columns per slab per partition (4096)
OSLOT = NPAIR * D         # psum/output columns per slab (4096)
MM_N = 512                # matmul free size (8 pairs x 64 j)
NROUND = 2                # rounds per slab (each fills half of PSUM)
MM_PER_SLAB = NROUND * NR * E   # 32
XBUFS = 4                 # input slab slots resident in SBUF
OBUFS = 4                 # output slab slots resident in SBUF


def build_nc(bs):
    import concourse.bass as bass
    import concourse.mybir as mybir

    f32 = mybir.dt.float32
    bf16 = mybir.dt.bfloat16
    nslab = bs // SLAB
    assert bs % SLAB == 0

    nc = bass.Bass()
    # x arrives pre-shuffled by the host into the exact SBUF slab layout
    # [slab, partition, pair, (v j)], so loads are plain contiguous copies
    # with 8KB-per-partition DMA elements (full descriptor line rate).
    x = nc.declare_dram_parameter("x", [nslab, 128, SLOT], bf16,
                                  isOutput=False)
    w = nc.declare_dram_parameter("w", [128, E * 128], bf16, isOutput=False)
    # Time-major result, stored sequentially; host permutes to [b, t, j].
    out = nc.declare_dram_parameter("out", [nslab, 128, OSLOT], bf16,
                                    isOutput=True)

    with (
        nc.sbuf_tensor([128, XBUFS * SLOT], bf16) as xt,
        nc.sbuf_tensor([128, OBUFS * OSLOT], bf16) as ot,
        nc.sbuf_tensor([128, E * 128], bf16) as wt,
        nc.psum_tensor([128, OSLOT], f32) as pt,
        nc.semaphore("w_sem") as w_sem,
        nc.semaphore("in0") as in0, nc.semaphore("in1") as in1,
        nc.semaphore("in2") as in2, nc.semaphore("in3") as in3,
        nc.semaphore("out0") as ou0, nc.semaphore("out1") as ou1,
        nc.semaphore("out2") as ou2, nc.semaphore("out3") as ou3,
        nc.semaphore("pe_sem") as pe_sem,
        nc.semaphore("act_sem") as act_sem,
        nc.semaphore("dve_sem") as dve_sem,
        nc.Block() as block,
    ):
        in_sems = [in0, in1, in2, in3]
        out_sems = [ou0, ou1, ou2, ou3]
        HALF = OSLOT // NROUND          # 2048 psum cols per round

        def x_slot(i):
            s0 = (i % XBUFS) * SLOT
            return xt[:, s0:s0 + SLOT]

        def o_slot(i):
            s0 = (i % OBUFS) * OSLOT
            return ot[:, s0:s0 + OSLOT]

        @block.sync
        def _(sync):
            sync.dma_start(wt[:, :], w[:, :]).then_inc(w_sem, 16)
            for i in range(nslab):
                if i >= XBUFS:
                    # slot consumed by matmuls of slab i-XBUFS; the same-sem
                    # wait also orders this slot's successive loads
                    sync.wait_ge(pe_sem, MM_PER_SLAB * (i - XBUFS + 1))
                    sync.wait_ge(in_sems[i % XBUFS], 16 * (i // XBUFS))
                sync.dma_start(x_slot(i), x[i]).then_inc(in_sems[i % XBUFS], 16)

        @block.tensor
        def _(tensor):
            tensor.wait_ge(w_sem, 16)
            for i in range(nslab):
                tensor.wait_ge(in_sems[i % XBUFS], 16 * (i // XBUFS + 1))
                rhs = x_slot(i).rearrange(
                    "p (pair v j) -> p v pair j", v=E, j=D)
                for c in range(NROUND):
                    if i >= 1:
                        # psum half recycled: previous slab's copies of this
                        # round must have drained it (ACT low half, DVE high)
                        tensor.wait_ge(act_sem, NROUND * (i - 1) + c + 1)
                        tensor.wait_ge(dve_sem, NROUND * (i - 1) + c + 1)
                    for v in range(E):
                        for r in range(NR):
                            # strips r stream concurrently in PE row bands
                            nc.tensor.matmul(
                                pt[:, c * HALF + r * MM_N:
                                   c * HALF + (r + 1) * MM_N],
                                wt[r * 32:(r + 1) * 32,
                                   v * 128:(v + 1) * 128],
                                rhs[r * 32:(r + 1) * 32, v,
                                    c * 8:(c + 1) * 8, :],
                                start=(v == 0), stop=(v == E - 1),
                                tile_position=(r * 32, 0),
                            ).then_inc(pe_sem, 1)

        @block.scalar
        def _(scalar):
            for i in range(nslab):
                for c in range(NROUND):
                    scalar.wait_ge(
                        pe_sem, MM_PER_SLAB * i + (c + 1) * NR * E)
                    if i >= OBUFS:
                        scalar.wait_ge(out_sems[i % OBUFS], 16 * (i // OBUFS))
                    nc.scalar.copy(
                        o_slot(i)[:, c * HALF:c * HALF + HALF // 2],
                        pt[:, c * HALF:c * HALF + HALF // 2],
                    ).then_inc(act_sem, 1)
                # the DMA trigger races the engine's own in-flight copy
                # writes, so even same-engine hand-off needs the sem
                scalar.wait_ge(act_sem, NROUND * (i + 1))
                scalar.wait_ge(dve_sem, NROUND * (i + 1))
                nc.scalar.dma_start(out[i], o_slot(i)).then_inc(
                    out_sems[i % OBUFS], 16)

        @block.vector
        def _(vector):
            for i in range(nslab):
                for c in range(NROUND):
                    vector.wait_ge(
                        pe_sem, MM_PER_SLAB * i + (c + 1) * NR * E)
                    if i >= OBUFS:
                        vector.wait_ge(out_sems[i % OBUFS], 16 * (i // OBUFS))
                    nc.vector.tensor_copy(
                        o_slot(i)[:, c * HALF + HALF // 2:(c + 1) * HALF],
                        pt[:, c * HALF + HALF // 2:(c + 1) * HALF],
                    ).then_inc(dve_sem, 1)

    return nc


def _step_matrices(F, Q, H, R, P0):
    """Host-side P/K recursion (float64). Returns per-step (A_t, B_t) with
    x_t = x_{t-1} @ A_t + y_t @ B_t."""
    d = F.shape[0]
    I = np.eye(d)
    Pm = P0.astype(np.float64)
    F64, Q64, H64, R64 = (m.astype(np.float64) for m in (F, Q, H, R))
    As, Bs = [], []
    for _ in range(T):
        Pm = F64 @ Pm @ F64.T + Q64
        S = H64 @ Pm @ H64.T + R64
        K = Pm @ H64.T @ np.linalg.inv(S)
        As.append(((I - K @ H64) @ F64).T)
        Bs.append(K.T)
        Pm = (I - K @ H64) @ Pm
    return As, Bs


def _scalar_gains(As, Bs):
    """If every A_t/B_t is c*I, return (a[T], b[T]) else None."""
    a, b = np.empty(T), np.empty(T)
    I = np.eye(D)
    for t in range(T):
        ca, cb = As[t][0, 0], Bs[t][0, 0]
        if not (np.allclose(As[t], ca * I, atol=1e-9) and
                np.allclose(Bs[t], cb * I, atol=1e-9)):
            return None
        a[t], b[t] = ca, cb
    return a, b


def _weight_matrix(a, b):
    W = np.zeros((T, T))
    for t in range(T):
        acc = 1.0
        W[t, t] = b[t]
        for s in range(t - 1, -1, -1):
            acc *= a[s + 1]
            W[t, s] = b[s] * acc
    return W.astype(np.float32)


def _weight_blocks(W):
    """Device weight tensor [128, E*128]: phase-v block of strip r holds the
    block-diagonal (over lanes q) lhsT with lhsT[(q,g), (q,t)] = W[t, g*E+v];
    identical [32, E*128] tiles replicated at rows 0/32/64/96."""
    wm = np.zeros((128, E * 128), dtype=np.float32)
    for v in range(E):
        blk = W[:, v::E].T          # [NG, T]: blk[g, t] = W[t, g*E+v]
        for q in range(2):
            wm[q * NG:(q + 1) * NG,
               v * 128 + q * 64:v * 128 + (q + 1) * 64] = blk
    for r in range(1, NR):
        wm[r * 32:(r + 1) * 32] = wm[:32]
    return wm.astype(ml_dtypes.bfloat16)


def _numpy_fallback(input_tensor, As, Bs, x0):
    """General-parameter path (never hit for the shipped inputs)."""
    y = input_tensor.astype(np.float32)
    x = np.broadcast_to(x0.astype(np.float32)[:, 0][None, :], (y.shape[0], D)).copy()
    out = np.empty_like(y)
    for t in range(T):
        x = x @ As[t].astype(np.float32) + y[:, t, :] @ Bs[t].astype(np.float32)
        out[:, t, :] = x
    return out


def device_args(input_tensor, wblk=None):
    """(nc, in_maps) for run_bass_kernel_spmd; input_tensor full fp32.

    Pre-shuffles the input into the device slab layout: slab i, partition
    p = r*32 + q*16 + g, columns (pair, v, j) with b = slab*128 +
    (r*BPAIR + pair)*2 + q and s = g*E + v."""
    if "nc" not in _CACHE:
        _CACHE["nc"] = build_nc(BS)
    nc = _CACHE["nc"]
    if wblk is None:
        wblk = _CACHE["wblk"]
    xb = np.ascontiguousarray(input_tensor).astype(ml_dtypes.bfloat16)
    nslab_full = B // SLAB
    xb = xb.reshape(nslab_full, NR, BPAIR, 2, NG, E, D)   # i r pair q g v j
    xb = np.ascontiguousarray(xb.transpose(0, 1, 3, 4, 2, 5, 6))
    xb = xb.reshape(nslab_full, 128, SLOT)
    nsc = BS // SLAB
    in_maps = [
        {"x": xb[i * nsc:(i + 1) * nsc], "w": wblk}
        for i in range(NCORES)
    ]
    return nc, in_maps


def _unpermute(res_core):
    """Device layout [nslab, 128, OSLOT] -> [BS, T, D] (still bf16).

    Partition dim is (q, t); columns are (round c, strip r, pair p, j) with
    batch b = slab*128 + (r*BPAIR + c*8 + p)*2 + q."""
    nslab = BS // SLAB
    v = res_core.reshape(nslab, 2, T, NROUND, NR, 8, D)
    v = v.transpose(0, 4, 3, 5, 1, 2, 6)     # (slab, r, c, p, q, t, j)
    return v.reshape(BS, T, D)


def _run_device(x_full, wblk):
    from concourse.bass_utils import run_bass_kernel_spmd

    nc, in_maps = device_args(x_full, wblk)
    res = run_bass_kernel_spmd(nc, in_maps, list(range(NCORES)))
    parts = [_unpermute(np.asarray(res.results[i]["out"]))
             for i in range(NCORES)]
    return np.concatenate(parts, axis=0).astype(np.float32)


def kernel(input_tensor, transition_matrix, transition_covariance,
           observation_matrix, observation_covariance,
           state_estimate, error_covariance):
    input_tensor = np.asarray(input_tensor, dtype=np.float32)
    F = np.asarray(transition_matrix, dtype=np.float32)
    Q = np.asarray(transition_covariance, dtype=np.float32)
    H = np.asarray(observation_matrix, dtype=np.float32)
    R = np.asarray(observation_covariance, dtype=np.float32)
    x0 = np.asarray(state_estimate, dtype=np.float32)
    P0 = np.asarray(error_covariance, dtype=np.float32)

    As, Bs = _step_matrices(F, Q, H, R, P0)
    sg = _scalar_gains(As, Bs)
    if sg is None:
        return _numpy_fallback(input_tensor, As, Bs, x0)

    a, b = sg
    W = _weight_matrix(a, b)
    wblk = _weight_blocks(W)
    _CACHE["wblk"] = wblk
    out = _run_device(input_tensor, wblk)

    if np.any(x0 != 0.0):
        alpha = np.cumprod(a).astype(np.float32)          # [T]
        out = out + alpha[None, :, None] * x0[:, 0][None, None, :]
    return out


# revision 19
# speedup vs baseline: 1.1234x; 1.1234x over previous
"""Batched Kalman filter for Trainium2 (Bass), 8-core data parallel.

The reference filter's P/K evolution is data- and batch-independent, so the
per-step gains can be computed on the host. When every per-step update matrix
is a scalar multiple of the identity (true for the shipped identity
parameters), the whole filter collapses to

    out[b] = W @ y[b]        W[t, s] = b_s * prod_{r=s+1..t} a_r   (lower-tri)

with a_t = 1 - k_t, b_t = k_t from the scalar gain recursion. On device this
is a single [64, 64] weight matmul applied per batch element.

This problem is HBM-bandwidth bound, so the kernel optimizes DMA traffic:

* Inputs/outputs move as bf16 (host converts; rel tolerance is 2e-2 and the
  bf16 round-trip costs ~0.7% rms), halving HBM bytes vs fp32.
* The host pre-shuffles the input (during the bf16-conversion pass it does
  anyway) into the exact SBUF slab layout, so every device load is a plain
  contiguous [128, 4096] copy with 8KB-per-partition DMA elements (full
  descriptor line rate; measured 512B elements only reach ~half rate).
* The time contraction is split into E=4 accumulating passes over PSUM with
  per-phase block-diagonal weights (a consequence of keeping E consecutive
  time-rows of one batch element in one partition). Each pass contracts only
  32 rows, so the PE array is row-tiled: the slab's four pair-blocks live in
  partition strips 0-31/32-63/64-95/96-127 and four 32-row matmuls stream
  concurrently in the four PE sub-array strips (explicit tile_position;
  partitions 96+ require it, base_partition() inference rejects them).
* The store is a straight sequential copy of the time-major result (8KB
  contiguous runs -> full bandwidth); the host undoes the layout permute
  during the gather/convert step it does anyway.
"""

import numpy as np
import ml_dtypes

B = 16384
NCORES = 8
BS = B // NCORES          # 2048 batch rows per core

T = 64
D = 64

E = 4                     # time-rows packed per DMA element (E*D*2 = 512B)
NG = T // E               # contraction groups per batch lane
NR = 4                    # PE row strips (pair-blocks per slab)

_CACHE = {}

SLAB = 128                # batch rows per slab
NPAIR = SLAB // 2         # batch pairs per slab
BPAIR = NPAIR // NR       # pairs per strip (16)
SLOT = BPAIR * E * D      # input columns per slab per partition (4096)
OSLOT = NPAIR * D         # psum/output columns per slab (4096)
MM_N = 512                # matmul free size (8 pairs x 64 j)
NROUND = 2                # rounds per slab (each fills half of PSUM)
MM_PER_SLAB = NROUND * NR * E   # 32
XBUFS = 4                 # input slab slots resident in SBUF
OBUFS = 4                 # output slab slots resident in SBUF


def build_nc(bs):
    import concourse.bass as bass
    import concourse.mybir as mybir

    f32 = mybir.dt.float32
    bf16 = mybir.dt.bfloat16
    nslab = bs // SLAB
    assert bs % SLAB == 0

    nc = bass.Bass()
    # x arrives pre-shuffled by the host into the exact SBUF slab layout
    # [slab, partition, pair, (v j)], so loads are plain contiguous copies
    # with 8KB-per-partition DMA elements (full descriptor line rate).
    x = nc.declare_dram_parameter("x", [nslab, 128, SLOT], bf16,
                                  isOutput=False)
    w = nc.declare_dram_parameter("w", [128, E * 128], bf16, isOutput=False)
    # Time-major result, stored sequentially; host permutes to [b, t, j].
    out = nc.declare_dram_parameter("out", [nslab, 128, OSLOT], bf16,
                                    isOutput=True)

    with (
        nc.sbuf_tensor([128, XBUFS * SLOT], bf16) as xt,
        nc.sbuf_tensor([128, OBUFS * OSLOT], bf16) as ot,
        nc.sbuf_tensor([128, E * 128], bf16) as wt,
        nc.psum_tensor([128, OSLOT], f32) as pt,
        nc.semaphore("w_sem") as w_sem,
        nc.semaphore("in0") as in0, nc.semaphore("in1") as in1,
        nc.semaphore("in2") as in2, nc.semaphore("in3") as in3,
        nc.semaphore("out0") as ou0, nc.semaphore("out1") as ou1,
        nc.semaphore("out2") as ou2, nc.semaphore("out3") as ou3,
        nc.semaphore("pe_sem") as pe_sem,
        nc.semaphore("act_sem") as act_sem,
        nc.semaphore("dve_sem") as dve_sem,
        nc.Block() as block,
    ):
        in_sems = [in0, in1, in2, in3]
        out_sems = [ou0, ou1, ou2, ou3]
        HALF = OSLOT // NROUND          # 2048 psum cols per round

        def x_slot(i):
            s0 = (i % XBUFS) * SLOT
            return xt[:, s0:s0 + SLOT]

        def o_slot(i):
            s0 = (i % OBUFS) * OSLOT
            return ot[:, s0:s0 + OSLOT]

        @block.sync
        def _(sync):
            sync.dma_start(wt[:, :], w[:, :]).then_inc(w_sem, 16)
            for i in range(nslab):
                if i >= XBUFS:
                    # slot consumed by matmuls of slab i-XBUFS; the same-sem
                    # wait also orders this slot's successive loads
                    sync.wait_ge(pe_sem, MM_PER_SLAB * (i - XBUFS + 1))
                    sync.wait_ge(in_sems[i % XBUFS], 16 * (i // XBUFS))
                sync.dma_start(x_slot(i), x[i]).then_inc(in_sems[i % XBUFS], 16)

        @block.tensor
        def _(tensor):
            tensor.wait_ge(w_sem, 16)
            for i in range(nslab):
                tensor.wait_ge(in_sems[i % XBUFS], 16 * (i // XBUFS + 1))
                rhs = x_slot(i).rearrange(
                    "p (pair v j) -> p v pair j", v=E, j=D)
                for c in range(NROUND):
                    if i >= 1:
                        # psum half recycled: previous slab's copies of this
                        # round must have drained it (ACT low half, DVE high)
                        tensor.wait_ge(act_sem, NROUND * (i - 1) + c + 1)
                        tensor.wait_ge(dve_sem, NROUND * (i - 1) + c + 1)
                    for v in range(E):
                        for r in range(NR):
                            # strips r stream concurrently in PE row bands
                            nc.tensor.matmul(
                                pt[:, c * HALF + r * MM_N:
                                   c * HALF + (r + 1) * MM_N],
                                wt[r * 32:(r + 1) * 32,
                                   v * 128:(v + 1) * 128],
                                rhs[r * 32:(r + 1) * 32, v,
                                    c * 8:(c + 1) * 8, :],
                                start=(v == 0), stop=(v == E - 1),
                                tile_position=(r * 32, 0),
                            ).then_inc(pe_sem, 1)

        @block.scalar
        def _(scalar):
            for i in range(nslab):
                for c in range(NROUND):
                    scalar.wait_ge(
                        pe_sem, MM_PER_SLAB * i + (c + 1) * NR * E)
                    if i >= OBUFS:
                        scalar.wait_ge(out_sems[i % OBUFS], 16 * (i // OBUFS))
                    nc.scalar.copy(
                        o_slot(i)[:, c * HALF:c * HALF + HALF // 2],
                        pt[:, c * HALF:c * HALF + HALF // 2],
                    ).then_inc(act_sem, 1)
                # the DMA trigger races the engine's own in-flight copy
                # writes, so even same-engine hand-off needs the sem
                scalar.wait_ge(act_sem, NROUND * (i + 1))
                scalar.wait_ge(dve_sem, NROUND * (i + 1))
                nc.scalar.dma_start(out[i], o_slot(i)).then_inc(
                    out_sems[i % OBUFS], 16)

        @block.vector
        def _(vector):
            for i in range(nslab):
                for c in range(NROUND):
                    vector.wait_ge(
                        pe_sem, MM_PER_SLAB * i + (c + 1) * NR * E)
                    if i >= OBUFS:
                        vector.wait_ge(out_sems[i % OBUFS], 16 * (i // OBUFS))
                    nc.vector.tensor_copy(
                        o_slot(i)[:, c * HALF + HALF // 2:(c + 1) * HALF],
                        pt[:, c * HALF + HALF // 2:(c + 1) * HALF],
                    ).then_inc(dve_sem, 1)

    return nc


def _step_matrices(F, Q, H, R, P0):
    """Host-side P/K recursion (float64). Returns per-step (A_t, B_t) with
    x_t = x_{t-1} @ A_t + y_t @ B_t."""
    d = F.shape[0]
    I = np.eye(d)
    Pm = P0.astype(np.float64)
    F64, Q64, H64, R64 = (m.astype(np.float64) for m in (F, Q, H, R))
    As, Bs = [], []
    for _ in range(T):
        Pm = F64 @ Pm @ F64.T + Q64
        S = H64 @ Pm @ H64.T + R64
        K = Pm @ H64.T @ np.linalg.inv(S)
        As.append(((I - K @ H64) @ F64).T)
        Bs.append(K.T)
        Pm = (I - K @ H64) @ Pm
    return As, Bs


def _scalar_gains(As, Bs):
    """If every A_t/B_t is c*I, return (a[T], b[T]) else None."""
    a, b = np.empty(T), np.empty(T)
    I = np.eye(D)
    for t in range(T):
        ca, cb = As[t][0, 0], Bs[t][0, 0]
        if not (np.allclose(As[t], ca * I, atol=1e-9) and
                np.allclose(Bs[t], cb * I, atol=1e-9)):
            return None
        a[t], b[t] = ca, cb
    return a, b


def _weight_matrix(a, b):
    W = np.zeros((T, T))
    for t in range(T):
        acc = 1.0
        W[t, t] = b[t]
        for s in range(t - 1, -1, -1):
            acc *= a[s + 1]
            W[t, s] = b[s] * acc
    return W.astype(np.float32)


def _weight_blocks(W):
    """Device weight tensor [128, E*128]: phase-v block of strip r holds the
    block-diagonal (over lanes q) lhsT with lhsT[(q,g), (q,t)] = W[t, g*E+v];
    identical [32, E*128] tiles replicated at rows 0/32/64/96."""
    wm = np.zeros((128, E * 128), dtype=np.float32)
    for v in range(E):
        blk = W[:, v::E].T          # [NG, T]: blk[g, t] = W[t, g*E+v]
        for q in range(2):
            wm[q * NG:(q + 1) * NG,
               v * 128 + q * 64:v * 128 + (q + 1) * 64] = blk
    for r in range(1, NR):
        wm[r * 32:(r + 1) * 32] = wm[:32]
    return wm.astype(ml_dtypes.bfloat16)


def _numpy_fallback(input_tensor, As, Bs, x0):
    """General-parameter path (never hit for the shipped inputs)."""
    y = input_tensor.astype(np.float32)
    x = np.broadcast_to(x0.astype(np.float32)[:, 0][None, :], (y.shape[0], D)).copy()
    out = np.empty_like(y)
    for t in range(T):
        x = x @ As[t].astype(np.float32) + y[:, t, :] @ Bs[t].astype(np.float32)
        out[:, t, :] = x
    return out


def device_args(input_tensor, wblk=None):
    """(nc, in_maps) for run_bass_kernel_spmd; input_tensor full fp32.

    Pre-shuffles the input into the device slab layout: slab i, partition
    p = r*32 + q*16 + g, columns (pair, v, j) with b = slab*128 +
    (r*BPAIR + pair)*2 + q and s = g*E + v."""
    if "nc" not in _CACHE:
        _CACHE["nc"] = build_nc(BS)
    nc = _CACHE["nc"]
    if wblk is None:
        wblk = _CACHE["wblk"]
    xb = np.ascontiguousarray(input_tensor).astype(ml_dtypes.bfloat16)
    nslab_full = B // SLAB
    xb = xb.reshape(nslab_full, NR, BPAIR, 2, NG, E, D)   # i r pair q g v j
    xb = np.ascontiguousarray(xb.transpose(0, 1, 3, 4, 2, 5, 6))
    xb = xb.reshape(nslab_full, 128, SLOT)
    nsc = BS // SLAB
    in_maps = [
        {"x": xb[i * nsc:(i + 1) * nsc], "w": wblk}
        for i in range(NCORES)
    ]
    return nc, in_maps


def _unpermute(res_core):
    """Device layout [nslab, 128, OSLOT] -> [BS, T, D] (still bf16).

    Partition dim is (q, t); columns are (round c, strip r, pair p, j) with
    batch b = slab*128 + (r*BPAIR + c*8 + p)*2 + q."""
    nslab = BS // SLAB
    v = res_core.reshape(nslab, 2, T, NROUND, NR, 8, D)
    v = v.transpose(0, 4, 3, 5, 1, 2, 6)     # (slab, r, c, p, q, t, j)
    return v.reshape(BS, T, D)


def _run_device(x_full, wblk):
    from concourse.bass_utils import run_bass_kernel_spmd

    nc, in_maps = device_args(x_full, wblk)
    res = run_bass_kernel_spmd(nc, in_maps, list(range(NCORES)))
    parts = [_unpermute(np.asarray(res.results[i]["out"]))
             for i in range(NCORES)]
    return np.concatenate(parts, axis=0).astype(np.float32)


def kernel(input_tensor, transition_matrix, transition_covariance,
           observation_matrix, observation_covariance,
           state_estimate, error_covariance):
    input_tensor = np.asarray(input_tensor, dtype=np.float32)
    F = np.asarray(transition_matrix, dtype=np.float32)
    Q = np.asarray(transition_covariance, dtype=np.float32)
    H = np.asarray(observation_matrix, dtype=np.float32)
    R = np.asarray(observation_covariance, dtype=np.float32)
    x0 = np.asarray(state_estimate, dtype=np.float32)
    P0 = np.asarray(error_covariance, dtype=np.float32)

    As, Bs = _step_matrices(F, Q, H, R, P0)
    sg = _scalar_gains(As, Bs)
    if sg is None:
        return _numpy_fallback(input_tensor, As, Bs, x0)

    a, b = sg
    W = _weight_matrix(a, b)
    wblk = _weight_blocks(W)
    _CACHE["wblk"] = wblk
    out = _run_device(input_tensor, wblk)

    if np.any(x0 != 0.0):
        alpha = np.cumprod(a).astype(np.float32)          # [T]
        out = out + alpha[None, :, None] * x0[:, 0][None, None, :]
    return out
